# revision 30
# baseline (speedup 1.0000x reference)
# Causal self-attention (B=4, T=2048, C=1024, 16 heads) on 8 NeuronCores.
# Sharding: core = (batch b = core//2) x (head-group hg = core%2, 8 heads each).
# Each core computes its 8 heads' attention for its batch plus the row-slice of
# the output projection; the host sums the two partial projections per batch.
#
# v4: host packs every DRAM tensor so each DMA moves 8KB-contiguous runs per
# SBUF partition (128 descriptors per ~1MB transfer -> HBM-roofline input).
# Chunk-major schedule: per 512-token query chunk, next chunk's qkv and the
# previous chunk's output projection are pumped into the ACT-bound attention
# loop as PE filler; S runs one jt ahead of AV; diagonal blocks skip their
# fully-masked columns; output is staged bf16 and shipped once per chunk.
#
# Self-contained: hardcodes shapes; builds + compiles the Bass program once.

import contextlib

import numpy as np
import ml_dtypes

import concourse.bass as bass
import concourse.tile as tile
from concourse import bacc, mybir
from concourse.bass import AP
from concourse.bass_utils import run_bass_kernel_spmd

F32 = mybir.dt.float32
BF16 = mybir.dt.bfloat16
EXP = mybir.ActivationFunctionType.Exp
IDENT = mybir.ActivationFunctionType.Identity

B, T, C = 4, 2048, 1024
NH, HS = 16, 64
NHPC = 8          # heads per core
D = NHPC * HS     # 512: per-core qkv width
NCORES = 8
TT = T // 128     # 16 token tiles
TC = T // 512     # 4 token chunks
CT = C // 128     # 8 contraction tiles
DT = D // 128     # 4 d-tiles of qT/kT (= head pairs)
VW = 68           # per-head stride in v tile: [ones | v(64) | pad3]
XCH = CT * 512    # 4096: packed x/y columns per chunk

_cache = {}


def _build():
    nc = bacc.Bacc("TRN2", target_bir_lowering=False, debug=False,
                   num_devices=NCORES)

    # all inputs host-packed to [128, *] with per-partition-contiguous runs
    xP = nc.dram_tensor("xP", [128, TC * XCH], BF16, kind="ExternalInput")
    wqP = nc.dram_tensor("wqP", [128, CT * D], BF16, kind="ExternalInput")
    wkP = nc.dram_tensor("wkP", [128, CT * D], BF16, kind="ExternalInput")
    wvP = nc.dram_tensor("wvP", [128, CT * D], BF16, kind="ExternalInput")
    wpP = nc.dram_tensor("wpP", [128, DT * C], BF16, kind="ExternalInput")
    bq = nc.dram_tensor("bq", [128, DT], F32, kind="ExternalInput")
    bk = nc.dram_tensor("bk", [128, DT], F32, kind="ExternalInput")
    bv1 = nc.dram_tensor("bv1", [1, D], F32, kind="ExternalInput")
    yP = nc.dram_tensor("yP", [128, TC * XCH], BF16, kind="ExternalOutput")

    with tile.TileContext(nc) as tc, contextlib.ExitStack() as ctx:
        cpool = ctx.enter_context(tc.tile_pool(name="consts", bufs=1))
        wpool = ctx.enter_context(tc.tile_pool(name="w", bufs=1))
        qkpool = ctx.enter_context(tc.tile_pool(name="qk", bufs=1))
        vpool = ctx.enter_context(tc.tile_pool(name="v", bufs=1))
        opool = ctx.enter_context(tc.tile_pool(name="oT", bufs=1))
        ptpool = ctx.enter_context(tc.tile_pool(name="pt", bufs=8))
        ypool = ctx.enter_context(tc.tile_pool(name="y", bufs=2))
        y1pool = ctx.enter_context(tc.tile_pool(name="y1", bufs=1))
        rvpool = ctx.enter_context(tc.tile_pool(name="rv", bufs=4))
        recpool = ctx.enter_context(tc.tile_pool(name="rec", bufs=2))
        avpool = ctx.enter_context(tc.tile_pool(name="avb", bufs=4))
        qkv_ps = ctx.enter_context(
            tc.tile_pool(name="qkv_ps", bufs=2, space="PSUM"))
        s_ps = ctx.enter_context(
            tc.tile_pool(name="s_ps", bufs=2, space="PSUM"))
        o_ps = ctx.enter_context(
            tc.tile_pool(name="o_ps", bufs=2, space="PSUM"))

        # ---- input DMAs: big contiguous transfers, both HWDGE rings ----
        wvw = wpool.tile([128, 128], BF16, tag="wvw", name="wvw")
        nc.sync.dma_start(wvw[:], wvP.ap()[:, 0:128])  # tiny: warmup feed
        wvt = wpool.tile([128, CT * D], BF16, tag="wv", name="wv")
        wqt = wpool.tile([128, CT * D], BF16, tag="wq", name="wq")
        wkt = wpool.tile([128, CT * D], BF16, tag="wk", name="wk")
        wpt = wpool.tile([128, DT * C], BF16, tag="wp", name="wp")
        xt = wpool.tile([128, TC * XCH], BF16, tag="x", name="x")

        # quarters so consumers start after ~0.25MB; sync feeds x0/wq + x1/x2,
        # scalar feeds wv/wk + x3/wp.  bv arrives tiny and is broadcast here.
        HW4 = CT * D // 4
        for i in range(4):
            nc.scalar.dma_start(wvt[:, i * HW4:(i + 1) * HW4],
                                wvP.ap()[:, i * HW4:(i + 1) * HW4])
            nc.sync.dma_start(xt[:, i * HW4:(i + 1) * HW4],
                              xP.ap()[:, i * HW4:(i + 1) * HW4])
        for i in range(4):
            nc.scalar.dma_start(wkt[:, i * HW4:(i + 1) * HW4],
                                wkP.ap()[:, i * HW4:(i + 1) * HW4])
            nc.sync.dma_start(wqt[:, i * HW4:(i + 1) * HW4],
                              wqP.ap()[:, i * HW4:(i + 1) * HW4])
        bqt = cpool.tile([128, DT], F32, tag="bq")
        bkt = cpool.tile([128, DT], F32, tag="bk")
        bvs = cpool.tile([1, D], F32, tag="bvs")
        bvt = cpool.tile([128, D], F32, tag="bv")
        nc.sync.dma_start(bqt[:], bq.ap())
        nc.scalar.dma_start(bkt[:], bk.ap())
        nc.scalar.dma_start(bvs[:], bv1.ap())
        nc.gpsimd.partition_broadcast(bvt[:, :], bvs[0:1, :])
        for cch, eng in ((1, nc.sync), (2, nc.sync), (3, nc.scalar)):
            eng.dma_start(xt[:, cch * XCH:(cch + 1) * XCH],
                          xP.ap()[:, cch * XCH:(cch + 1) * XCH])
        nc.scalar.dma_start(wpt[:], wpP.ap())

        def xap(ct, a, b):
            # x columns [a,b) of contraction tile ct; [a,b) within one chunk
            c = a // 512
            col = c * XCH + ct * 512 + (a - c * 512)
            return xt[:, col:col + (b - a)]

        # ---- one-time consts ----
        ones8 = cpool.tile([128, NHPC], F32, tag="ones8")
        nc.vector.memset(ones8[:], 1.0)
        mf = cpool.tile([128, 128], F32, tag="mf", name="mf")
        nc.vector.memset(mf[:], 1.0)
        nc.gpsimd.affine_select(
            mf[:], mf[:], pattern=[[1, 128]],
            compare_op=mybir.AluOpType.is_ge, fill=0.0,
            base=0, channel_multiplier=-1)
        mstrip = cpool.tile([128, 128], BF16, tag="mstrip", name="mstrip")
        nc.vector.tensor_copy(mstrip[:], mf[:])
        # preload the exp table set while DMAs run (first real exp would
        # otherwise pay the ~2.7us ACT_TABLE_LOAD on the critical path)
        expw = cpool.tile([1, 8], BF16, tag="expw", name="expw")
        nc.scalar.activation(expw[:], ones8[0:1, 0:8], EXP)

        # ---- warm-up matmuls on a tiny early tile: ~4.3us of sustained PE
        # activity so the HAM clock-gate reaches 8/8 before the real work
        warm_ps = qkv_ps.tile([128, 512], F32, tag="qkv", name="warmps")
        for w in range(48):
            nc.tensor.matmul(
                warm_ps[:, 0:128], wvw[:], wvw[:],
                start=True, stop=True, skip_group_check=True)

        # ---- persistent tiles ----
        vt = [vpool.tile([128, NHPC * VW], BF16, tag=f"v{j}", name=f"v{j}")
              for j in range(TT)]
        qT = [qkpool.tile([128, T], BF16, tag=f"q{d}", name=f"q{d}")
              for d in range(DT)]
        kT = [qkpool.tile([128, T], BF16, tag=f"k{d}", name=f"k{d}")
              for d in range(DT)]
        oT = [opool.tile([128, T], BF16, tag=f"oT{d}", name=f"oT{d}")
              for d in range(DT)]

        # ---- filler generators: yield once per emitted matmul ----
        def gen_v(j):
            ps = qkv_ps.tile([128, D], F32, tag="qkv", name="qkvps")
            for ct in range(CT):
                nc.tensor.matmul(
                    ps[:], xap(ct, j * 128, (j + 1) * 128),
                    wvt[:, ct * D:(ct + 1) * D],
                    start=(ct == 0), stop=(ct == CT - 1))
                yield
            vre = vt[j][:].rearrange("p (h x) -> p h x", h=NHPC)
            nc.vector.tensor_copy(
                vre[:, :, 0:1], ones8[:].rearrange("p (h x) -> p h x", x=1))
            nc.vector.tensor_add(
                vre[:, :, 1:65],
                ps[:].rearrange("p (h x) -> p h x", h=NHPC),
                bvt[:].rearrange("p (h x) -> p h x", h=NHPC))

        def gen_qk(hp, c, which):
            wt_, bt_, out = ((wqt, bqt, qT), (wkt, bkt, kT))[which]
            ps = qkv_ps.tile([128, 512], F32, tag="qkv", name="qkvps")
            for ct in range(CT):
                nc.tensor.matmul(
                    ps[:], wt_[:, ct * D + hp * 128: ct * D + (hp + 1) * 128],
                    xap(ct, c * 512, (c + 1) * 512),
                    start=(ct == 0), stop=(ct == CT - 1))
                yield
            nc.vector.tensor_scalar_add(
                out[hp][:, c * 512:(c + 1) * 512], ps[:], bt_[:, hp:hp + 1])

        def gen_proj(c, hps=(0, 1, 2, 3), ysrc=None):
            # proj over head pairs `hps`; adds ysrc (bf16 partial) if given;
            # ships the whole chunk in one DMA when it's the final pass.
            final = hps[-1] == DT - 1
            ys = ypool.tile([128, XCH], BF16, tag="y", name="ys") if final \
                else y1pool.tile([128, XCH], BF16, tag="y1", name="ys1")
            for o in range(CT):
                ps = qkv_ps.tile([128, 512], F32, tag="qkv", name="qkvps")
                for i, hp in enumerate(hps):
                    nc.tensor.matmul(
                        ps[:], wpt[:, hp * C + o * 128: hp * C + (o + 1) * 128],
                        oT[hp][:, c * 512:(c + 1) * 512],
                        start=(i == 0), stop=(i == len(hps) - 1))
                    yield
                osl = slice(o * 512, (o + 1) * 512)
                if ysrc is None:
                    nc.vector.tensor_copy(ys[:, osl], ps[:])
                else:
                    nc.vector.tensor_add(ys[:, osl], ps[:], ysrc[:, osl])
                if final and o == CT // 2 - 1:
                    nc.scalar.dma_start(
                        yP.ap()[:, c * XCH:c * XCH + XCH // 2],
                        ys[:, 0:XCH // 2])
            if final:
                nc.scalar.dma_start(
                    yP.ap()[:, c * XCH + XCH // 2:(c + 1) * XCH],
                    ys[:, XCH // 2:])
            else:
                gen_proj.partial = ys

        # two filler queues: F (projections for next chunk; must finish
        # before that chunk's attention) and P (output proj; can linger)
        fillF = []
        fillP = []

        def pump(n):
            for _ in range(n):
                q = fillF if fillF else fillP
                if not q:
                    return
                try:
                    next(q[0])
                except StopIteration:
                    q.pop(0)

        def drain(q):
            while q:
                try:
                    next(q[0])
                except StopIteration:
                    q.pop(0)

        # ---- attention block for head pair hp, query chunk c ----
        # Diagonal j-tiles (jt = 4c+t) skip their fully-masked first 128t
        # columns in S, exp and AV; only the 128-col triangle strip is masked.
        def attention(hp, c, rate):
            njt = 4 * c + 4
            op0 = o_ps.tile([128, 512], F32, tag="o", name="ops")
            op1 = o_ps.tile([128, 512], F32, tag="o", name="ops")
            sps = {}

            def emit_S(jt):
                lo = 128 * (jt - 4 * c) if jt >= 4 * c else 0
                sp = s_ps.tile([128, 1024], F32, tag="s", name="sps")
                for half in range(2):
                    nc.tensor.matmul(
                        sp[:, half * 512 + lo:(half + 1) * 512],
                        kT[hp][half * 64:(half + 1) * 64,
                               jt * 128:(jt + 1) * 128],
                        qT[hp][half * 64:(half + 1) * 64,
                               c * 512 + lo:(c + 1) * 512],
                        start=True, stop=True)
                sps[jt] = sp

            emit_S(0)
            pcr = 0.0
            for jt in range(njt):
                if jt + 1 < njt:
                    emit_S(jt + 1)
                sp = sps.pop(jt)
                t = jt - 4 * c
                lo = 128 * t if t >= 0 else 0
                pt = ptpool.tile([128, 1024], BF16, tag="pt", name="pt")
                nc.scalar.activation(pt[:, lo:1024], sp[:, lo:1024],
                                     EXP, scale=0.125)
                if t >= 0:
                    nc.vector.tensor_mul(
                        pt[:, lo:lo + 128], pt[:, lo:lo + 128], mstrip[:])
                    nc.gpsimd.affine_select(
                        pt[:, 512 + lo:512 + lo + 128],
                        pt[:, 512 + lo:512 + lo + 128],
                        pattern=[[1, 128]],
                        compare_op=mybir.AluOpType.is_ge, fill=0.0,
                        base=0, channel_multiplier=-1)
                pcr += rate
                if pcr >= 1.0:
                    k = int(pcr)
                    pump(k)       # fillers land in the exp-wait window
                    pcr -= k
                for half, op in ((0, op0), (1, op1)):
                    h = 2 * hp + half
                    nc.tensor.matmul(
                        op[0:65, lo:512], vt[jt][:, h * VW:h * VW + 65],
                        pt[:, half * 512 + lo:(half + 1) * 512],
                        start=(jt == 0), stop=(jt == njt - 1))

            # normalization: stage unnormalized rows out of PSUM (releases
            # the bank), recip the rowsum straight from PSUM partition 0,
            # DMA-shift 1:65 -> oT, then scale oT in place (aligned).
            cs = slice(c * 512, (c + 1) * 512)
            for half, op, dst in ((0, op0, oT[hp][0:64, cs]),
                                  (1, op1, oT[hp][64:128, cs])):
                av = avpool.tile([128, 512], BF16, tag="avb", name="avb")
                nc.vector.tensor_copy(av[0:65, :], op[0:65, :])
                rec = recpool.tile([1, 512], F32, tag="rec", name="rec")
                nc.vector.reciprocal_approx_fast(rec[:], op[0:1, :])
                rv = rvpool.tile([128, 512], F32, tag="rv", name="rv")
                nc.gpsimd.partition_broadcast(rv[:, :], rec[0:1, :])
                nc.sync.dma_start(dst, av[1:65, :])
                nc.vector.tensor_mul(
                    dst, dst, rv[half * 64:(half + 1) * 64, :])

        # ---- main schedule: chunk-major ----
        # F_0 emitted directly (nothing to interleave with yet)
        for j in range(4):
            for _ in gen_v(j):
                pass
        for hp in range(DT):
            for which in range(2):
                for _ in gen_qk(hp, 0, which):
                    pass

        rates = [3.0, 1.0, 0.7, 0.9]
        for c in range(TC):
            if c + 1 < TC:
                for j in range(4 * (c + 1), 4 * (c + 1) + 4):
                    fillF.append(gen_v(j))
                for hp in range(DT):
                    for which in range(2):
                        fillF.append(gen_qk(hp, c + 1, which))
            for hp in range(DT):
                attention(hp, c, rates[c])
                if c == TC - 1 and hp == 2:
                    fillF.append(gen_proj(c, hps=(0, 1, 2)))
            drain(fillF)
            if c != TC - 2:
                drain(fillP)          # finish old proj before queueing new
            if c < TC - 1:
                fillP.append(gen_proj(c))
        drain(fillP)                  # proj(2) remnants cover the last norm
        for _ in gen_proj(TC - 1, hps=(3,), ysrc=gen_proj.partial):
            pass

    nc.compile()
    return nc


def _shard_inputs(x, Wk, bk, Wq, bq, Wv, bv, Wp, bp):
    bf = ml_dtypes.bfloat16

    def packw(W, sl):  # [C, D-slice] -> [128, CT*D] partition-contiguous
        return np.ascontiguousarray(
            W[:, sl].reshape(CT, 128, D).transpose(1, 0, 2)
            .reshape(128, CT * D)).astype(bf)

    in_maps = []
    for core in range(NCORES):
        b, hg = core // 2, core % 2
        sl = slice(hg * D, (hg + 1) * D)
        xb = np.asarray(x[b], np.float32)  # [T, C]
        xp = (xb.T.reshape(CT, 128, TC, 512).transpose(1, 2, 0, 3)
              .reshape(128, TC * XCH))
        wpp = (Wp[sl, :].reshape(DT, 128, C).transpose(1, 0, 2)
               .reshape(128, DT * C))
        in_maps.append({
            "xP": np.ascontiguousarray(xp).astype(bf),
            "wqP": packw(Wq, sl),
            "wkP": packw(Wk, sl),
            "wvP": packw(Wv, sl),
            "wpP": np.ascontiguousarray(wpp).astype(bf),
            "bq": np.ascontiguousarray(
                bq[sl].reshape(DT, 128).T).astype(np.float32),
            "bk": np.ascontiguousarray(
                bk[sl].reshape(DT, 128).T).astype(np.float32),
            "bv1": np.ascontiguousarray(
                bv[sl].reshape(1, D)).astype(np.float32),
        })
    return in_maps


def kernel(x, Wk, bk, Wq, bq, Wv, bv, Wp, bp, _trace=False, _trace_kwargs=None):
    x, Wk, bk, Wq, bq, Wv, bv, Wp, bp = [
        np.asarray(a) for a in (x, Wk, bk, Wq, bq, Wv, bv, Wp, bp)]
    if "nc" not in _cache:
        _cache["nc"] = _build()
    nc = _cache["nc"]
    in_maps = _shard_inputs(x, Wk, bk, Wq, bq, Wv, bv, Wp, bp)
    kw = dict(_trace_kwargs or {})
    res = run_bass_kernel_spmd(nc, in_maps, core_ids=list(range(NCORES)),
                               trace=_trace, **kw)
    out = np.empty((B, T, C), np.float32)
    for b in range(B):
        yp = (res.results[2 * b]["yP"].astype(np.float32)
              + res.results[2 * b + 1]["yP"].astype(np.float32))
        # yP[p, c*XCH + o*512 + d] = y_partial[o*128+p, c*512+d]
        yp = (yp.reshape(128, TC, CT, 512).transpose(2, 0, 1, 3)
              .reshape(C, T))
        out[b] = yp.T + bp[None, :]
    if _trace:
        _cache["last_results"] = res
    return out
